# revision 7
# baseline (speedup 1.0000x reference)
"""Conv2d(128->256, 3x3, pad 1, stride 1) on 32x56x56 fp32, for 8 trn2 cores.

Strategy: data-parallel over batch N=32 -> 4 images/core. Per core an
implicit-GEMM conv: C_in=128 is the partition (contraction) dim; for each
(kh, kw) tap a [128ci x 128co] weight tile multiplies a shifted window of the
column-padded input image held in SBUF, accumulating into PSUM over the 9 taps.
Output rows are processed in chunks of 8 (free dim 8*56=448 <= 512 PSUM bank).
Matmuls run in float16 (fp16 keeps ~2.6e-4 rel err) with fp32 PSUM accumulate.

Layout details (all tuned from perfetto traces):
- Two SBUF copies of each input slice: copy A with the row interior at column
  1 (serves kw=0 and kw=2 taps) and copy B at column 2 (serves kw=1). This
  keeps every matmul rhs at an even fp16 element offset; odd offsets cost ~18
  extra PE cycles per matmul (SBUF word-split reads).
- No zero pad ROWS: boundary chunks instead shrink the kh taps that would
  read them (448 -> 392 free, PSUM sub-range), saving ~1.1us of PE work.
  Copy A keeps zero pad COLUMNS (elems 0 and 57 of each 58-elem row).
- PE warmup: dummy matmuls opened ASAP (wu memset on the gpsimd queue, which
  starts ~1us before vector) keep the PE busy while head DMAs land, so the
  HAM clock gate (opens ~6us after first PE activity, half clock until then)
  is already open when the real stream runs.
- Head DMAs: first x slice is only 9 rows (enough for chunk 0) and the tap-
  first weight block is a separate small DMA, so the first real matmul's
  dependencies land as early as possible.
- Outputs: full-half bulk DMAs for images 0..2 (12.5KB/partition
  descriptors), but CHUNK-wise for the whole last image so the final chunk's
  writeout is not queued behind a 1.6MB bulk transfer in the DMA FIFOs.
"""
import numpy as np
from contextlib import ExitStack

N_FULL, C_IN, H, W = 32, 128, 56, 56
C_OUT, KS = 256, 3
N_CORES = 8
N_PER = N_FULL // N_CORES          # 4 images per core
PIX = H * W                         # 3136
ROWS = 8                            # output rows per psum chunk
RC = H // ROWS                      # 7 chunks
NF = ROWS * W                       # 448 free elems per matmul
NARROW = (ROWS - 1) * W             # 392 free elems for boundary taps
SW = 58                             # padded row stride (56 + 2 border cols)

T_ROWS = 33                         # top tile: x rows 0..32 (chunks 0-3)
B_ROWS = 25                         # bottom tile: x rows 31..55 (chunks 4-6)
B0 = 31                             # first x row held in the bottom tile
XT_A = 9                            # first sub-DMA: x rows 0..8 (chunk 0)
XT_B = 12                           # second: x rows 9..20
XT_C = T_ROWS - XT_A - XT_B         # third: x rows 21..32

# tap order per chunk kind: (kh, kw) tuples. start=first, stop=last.
# narrow taps (boundary rows) must not be first or last.
TAPS_MID = [(0, 0), (0, 1), (0, 2), (1, 0), (1, 1), (1, 2), (2, 0), (2, 1), (2, 2)]
TAPS_TOP = [(1, 0), (1, 1), (1, 2), (0, 0), (0, 1), (0, 2), (2, 0), (2, 1), (2, 2)]
TAPS_BOT = [(0, 0), (0, 1), (0, 2), (2, 0), (2, 1), (2, 2), (1, 0), (1, 1), (1, 2)]

_CACHE = {}


def _build():
    import concourse.tile as tile
    from concourse import mybir, bacc

    f32 = mybir.dt.float32
    f16 = mybir.dt.float16

    nc = bacc.Bacc("TRN2", target_bir_lowering=False, debug=False)
    x_d = nc.dram_tensor("x", [N_PER, C_IN, PIX], f16, kind="ExternalInput").ap()
    # host-pretransposed: [ci, half, k, co_half] (half-major, contiguous per
    # half); within each half tap k=3 (the first tap chunk 0 runs) is stored
    # first: host order TAPS_TOP.
    w_d = nc.dram_tensor("w", [C_IN, 2, KS * KS, 128], f16, kind="ExternalInput").ap()
    b_d = nc.dram_tensor("b", [C_OUT], f32, kind="ExternalInput").ap()
    y_d = nc.dram_tensor("y", [N_PER, C_OUT, PIX], f32, kind="ExternalOutput").ap()

    # host-order index of tap (kh, kw) inside a weight half (TAPS_TOP order)
    widx = {t: i for i, t in enumerate(TAPS_TOP)}

    with tile.TileContext(nc) as tc:
        with ExitStack() as ctx:
            wp = ctx.enter_context(tc.tile_pool(name="wp", bufs=1))
            xrawta = ctx.enter_context(tc.tile_pool(name="xrawta", bufs=1))
            xrawtb = ctx.enter_context(tc.tile_pool(name="xrawtb", bufs=1))
            xrawtc = ctx.enter_context(tc.tile_pool(name="xrawtc", bufs=1))
            xrawb = ctx.enter_context(tc.tile_pool(name="xrawb", bufs=1))
            xpadt = ctx.enter_context(tc.tile_pool(name="xpadt", bufs=4))
            xpadb = ctx.enter_context(tc.tile_pool(name="xpadb", bufs=4))
            pp = ctx.enter_context(tc.tile_pool(name="pp", bufs=4, space="PSUM"))
            op = ctx.enter_context(tc.tile_pool(name="op", bufs=2))

            # PE warmup: the HAM clock gate opens ~6us after the first PE
            # instruction; keep the PE busy with dummies until real work is
            # ready (~8.5us). wu memset goes on the gpsimd queue (earliest to
            # start); dummies shrink toward the end for a fine-grained handoff.
            wu = wp.tile([128, 448], f16)
            nc.gpsimd.memset(wu[:], 0.0)
            wups = pp.tile([128, NF], f32, tag="ps")
            for _ in range(6):
                nc.tensor.matmul(wups[:], wu[:, 0:128], wu[:], start=True, stop=True)
            for _ in range(8):
                nc.tensor.matmul(wups[:, 0:112], wu[:, 0:128], wu[:, 0:112], start=True, stop=True)

            # Weight half 0 in two pieces on the ACT ring: the first three
            # taps' weights (96KB) gate the first matmuls; the rest follows.
            w_r = wp.tile([C_IN, 2 * KS * KS * 128], f16)
            w_r4 = w_r[:].rearrange("p (h k co) -> p h k co", h=2, k=KS * KS)
            nc.scalar.dma_start(
                w_r4[:, 0, 0:3], w_d[:, 0, 0:3].rearrange("ci k co -> ci (k co)")
            )
            nc.scalar.dma_start(
                w_r4[:, 0, 3:9], w_d[:, 0, 3:9].rearrange("ci k co -> ci (k co)")
            )

            bias_sb = wp.tile([128, 2], f32)

            for n in range(N_PER):
                # top interior in three slices so chunk 0 unblocks early
                xrta = xrawta.tile([C_IN, XT_A * W], f16)
                nc.sync.dma_start(xrta[:], x_d[n, :, 0 : XT_A * W])
                xrtb = xrawtb.tile([C_IN, XT_B * W], f16)
                nc.sync.dma_start(xrtb[:], x_d[n, :, XT_A * W : (XT_A + XT_B) * W])
                xrtc = xrawtc.tile([C_IN, XT_C * W], f16)
                nc.sync.dma_start(xrtc[:], x_d[n, :, (XT_A + XT_B) * W : T_ROWS * W])
                xrb = xrawb.tile([C_IN, B_ROWS * W], f16)
                nc.sync.dma_start(xrb[:], x_d[n, :, B0 * W : (B0 + B_ROWS) * W])

                if n == 0:
                    # now that image-0's loads are queued: weight half 1 + bias
                    nc.scalar.dma_start(
                        w_r4[:, 1], w_d[:, 1].rearrange("ci k co -> ci (k co)")
                    )
                    nc.scalar.dma_start(bias_sb[:], b_d.rearrange("(h p) -> p h", h=2))

                # copy A: interior at col 1 (kw=0,2 taps; even rhs offsets
                # 0/2); zero border cols 0 and 57.  copy B: interior at col 2
                # (kw=1 taps; even rhs offset 2), no borders needed.
                xptA = xpadt.tile([C_IN, T_ROWS * SW], f16)
                xptA3 = xptA[:].rearrange("p (a b) -> p a b", a=T_ROWS)
                xptB = xpadt.tile([C_IN, T_ROWS * SW], f16)
                xptB3 = xptB[:].rearrange("p (a b) -> p a b", a=T_ROWS)
                nc.vector.memset(xptA3[:, :, 0:1], 0.0)
                nc.vector.memset(xptA3[:, :, SW - 1 : SW], 0.0)
                # interleave A/B per slice: the vector queue is in-order and
                # chunk 0 needs both A and B of the first slice ASAP
                for (src, a0, a1) in (
                    (xrta, 0, XT_A),
                    (xrtb, XT_A, XT_A + XT_B),
                    (xrtc, XT_A + XT_B, T_ROWS),
                ):
                    src3 = src[:].rearrange("p (a b) -> p a b", a=a1 - a0)
                    nc.vector.tensor_copy(xptA3[:, a0:a1, 1 : 1 + W], src3)
                    nc.vector.tensor_copy(xptB3[:, a0:a1, 2 : 2 + W], src3)

                xpbA = xpadb.tile([C_IN, B_ROWS * SW], f16)
                xpbA3 = xpbA[:].rearrange("p (a b) -> p a b", a=B_ROWS)
                xpbB = xpadb.tile([C_IN, B_ROWS * SW], f16)
                xpbB3 = xpbB[:].rearrange("p (a b) -> p a b", a=B_ROWS)
                nc.vector.memset(xpbA3[:, :, 0:1], 0.0)
                nc.vector.memset(xpbA3[:, :, SW - 1 : SW], 0.0)
                xrb3 = xrb[:].rearrange("p (a b) -> p a b", a=B_ROWS)
                nc.vector.tensor_copy(xpbA3[:, :, 1 : 1 + W], xrb3)
                nc.vector.tensor_copy(xpbB3[:, :, 2 : 2 + W], xrb3)

                out_sb = op.tile([128, 2 * PIX], f32)
                last_img = n == N_PER - 1
                for half in range(2):
                    for rc in range(RC):
                        ps = pp.tile([128, NF], f32)
                        taps = TAPS_TOP if rc == 0 else (TAPS_BOT if rc == RC - 1 else TAPS_MID)
                        for i, (kh, kw) in enumerate(taps):
                            lhsT = w_r4[:, half, widx[(kh, kw)], :]
                            # x rows needed: output rows rc*8..rc*8+7 read
                            # x rows rc*8+kh-1 .. rc*8+kh+6 (clipped at 0/55)
                            r0 = rc * ROWS + kh - 1
                            narrow_top = rc == 0 and kh == 0       # skip out row 0
                            narrow_bot = rc == RC - 1 and kh == 2  # skip out row 55
                            if rc < 4:
                                A3, B3, base = xptA3, xptB3, 0
                            else:
                                A3, B3, base = xpbA3, xpbB3, B0
                            src3 = B3 if kw == 1 else A3
                            coff = 2 if kw == 1 else kw
                            if narrow_top:
                                rhs = src3[:, 0:ROWS - 1, coff : coff + W]
                                dst = ps[:, W:NF]
                            elif narrow_bot:
                                lr = r0 - base
                                rhs = src3[:, lr : lr + ROWS - 1, coff : coff + W]
                                dst = ps[:, 0:NARROW]
                            else:
                                lr = r0 - base
                                rhs = src3[:, lr : lr + ROWS, coff : coff + W]
                                dst = ps[:]
                            nc.tensor.matmul(
                                dst, lhsT, rhs,
                                start=(i == 0), stop=(i == KS * KS - 1),
                            )
                        # psum -> sbuf with per-channel bias add
                        lo = half * PIX + rc * NF
                        if last_img:
                            # last image: write every chunk out as produced so
                            # nothing bulky sits ahead of the tail in the DMA
                            # queues; final chunk in two halves on two rings.
                            eng = nc.sync if half == 0 else nc.gpsimd
                            if half == 1 and rc == RC - 1:
                                HNF = NF // 2
                                for piece in range(2):
                                    po = lo + piece * HNF
                                    nc.vector.tensor_scalar_add(
                                        out_sb[:, po : po + HNF],
                                        ps[:, piece * HNF : (piece + 1) * HNF],
                                        bias_sb[:, half : half + 1],
                                    )
                                    peng = nc.sync if piece == 0 else nc.gpsimd
                                    peng.dma_start(
                                        y_d[n, 128:256, rc * NF + piece * HNF : rc * NF + (piece + 1) * HNF],
                                        out_sb[:, po : po + HNF],
                                    )
                                continue
                            nc.vector.tensor_scalar_add(
                                out_sb[:, lo : lo + NF],
                                ps[:],
                                bias_sb[:, half : half + 1],
                            )
                            eng.dma_start(
                                y_d[n, half * 128 : (half + 1) * 128, rc * NF : (rc + 1) * NF],
                                out_sb[:, lo : lo + NF],
                            )
                            continue
                        nc.vector.tensor_scalar_add(
                            out_sb[:, lo : lo + NF],
                            ps[:],
                            bias_sb[:, half : half + 1],
                        )
                    if not last_img:
                        eng = nc.scalar if half == 0 else nc.gpsimd
                        eng.dma_start(
                            y_d[n, half * 128 : (half + 1) * 128, :],
                            out_sb[:, half * PIX : (half + 1) * PIX],
                        )
    nc.compile()
    return nc


def _get_nc():
    if "nc" not in _CACHE:
        _CACHE["nc"] = _build()
    return _CACHE["nc"]


def _prep_inputs(x, weight, bias):
    # fp16 on host: halves input DMA bytes and drops the on-device casts
    x = np.ascontiguousarray(
        np.asarray(x, dtype=np.float32).astype(np.float16).reshape(N_FULL, C_IN, PIX)
    )
    # [co, ci, kh, kw] -> [ci, half, k, co_half], half-major; taps within a
    # half stored in TAPS_TOP order so chunk-0's first weights DMA first.
    w4 = (
        np.transpose(np.asarray(weight, dtype=np.float32), (1, 2, 3, 0))
        .reshape(C_IN, KS * KS, 2, 128)
        .transpose(0, 2, 1, 3)
    )  # [ci, half, k(row-major), co]
    perm = [kh * KS + kw for (kh, kw) in TAPS_TOP]
    w_t = np.ascontiguousarray(w4[:, :, perm, :].astype(np.float16))
    b = np.ascontiguousarray(bias, dtype=np.float32)
    return x, w_t, b


def kernel(x, weight, bias):
    from concourse.bass_utils import run_bass_kernel_spmd

    x, w_t, b = _prep_inputs(x, weight, bias)
    nc = _get_nc()
    in_maps = [
        {"x": x[i * N_PER : (i + 1) * N_PER], "w": w_t, "b": b}
        for i in range(N_CORES)
    ]
    res = run_bass_kernel_spmd(nc, in_maps, list(range(N_CORES)))
    y = np.concatenate(
        [res.results[i]["y"].reshape(N_PER, C_OUT, H, W) for i in range(N_CORES)],
        axis=0,
    )
    return y


# revision 13
# speedup vs baseline: 1.0567x; 1.0567x over previous
"""Conv2d(128->256, 3x3, pad 1, stride 1) on 32x56x56 fp32, for 8 trn2 cores.

Strategy: data-parallel over batch N=32 -> 4 images/core. Per core an
implicit-GEMM conv: C_in=128 is the partition (contraction) dim; for each
(kh, kw) tap a [128ci x 128co] weight tile multiplies a shifted window of the
column-padded input image held in SBUF, accumulating into PSUM over the 9 taps.
Output rows are processed in chunks of 8 (free dim 8*56=448 <= 512 PSUM bank).
Matmuls run in float16 (fp16 keeps ~2.6e-4 rel err) with fp32 PSUM accumulate.

Layout details (all tuned from perfetto traces):
- Two SBUF copies of each input slice: copy A with the row interior at column
  1 (serves kw=0 and kw=2 taps) and copy B at column 2 (serves kw=1). This
  keeps every matmul rhs at an even fp16 element offset; odd offsets cost ~18
  extra PE cycles per matmul (SBUF word-split reads).
- No zero pad ROWS: boundary chunks instead shrink the kh taps that would
  read them (448 -> 392 free, PSUM sub-range), saving ~1.1us of PE work.
  Copy A keeps zero pad COLUMNS (elems 0 and 57 of each 58-elem row).
- PE warmup: dummy matmuls opened ASAP (wu memset on the gpsimd queue, which
  starts ~1us before vector) keep the PE busy while head DMAs land, so the
  HAM clock gate (opens ~6us after first PE activity, half clock until then)
  is already open when the real stream runs.
- Head DMAs: first x slice is only 9 rows (enough for chunk 0) and the tap-
  first weight block is a separate small DMA, so the first real matmul's
  dependencies land as early as possible.
- Outputs: full-half bulk DMAs for images 0..2 (12.5KB/partition
  descriptors), but CHUNK-wise for the whole last image so the final chunk's
  writeout is not queued behind a 1.6MB bulk transfer in the DMA FIFOs.
"""
import numpy as np
from contextlib import ExitStack

N_FULL, C_IN, H, W = 32, 128, 56, 56
C_OUT, KS = 256, 3
N_CORES = 8
N_PER = N_FULL // N_CORES          # 4 images per core
PIX = H * W                         # 3136
ROWS = 8                            # output rows per psum chunk
RC = H // ROWS                      # 7 chunks
NF = ROWS * W                       # 448 free elems per matmul
NARROW = (ROWS - 1) * W             # 392 free elems for boundary taps
SW = 58                             # padded row stride (56 + 2 border cols)

T_ROWS = 33                         # top tile: x rows 0..32 (chunks 0-3)
B_ROWS = 25                         # bottom tile: x rows 31..55 (chunks 4-6)
B0 = 31                             # first x row held in the bottom tile
XT_A = 9                            # first sub-DMA: x rows 0..8 (chunk 0)
XT_B = 12                           # second: x rows 9..20
XT_C = T_ROWS - XT_A - XT_B         # third: x rows 21..32

# tap order per chunk kind: (kh, kw) tuples. start=first, stop=last.
# narrow taps (boundary rows) must not be first or last.
TAPS_MID = [(0, 0), (0, 1), (0, 2), (1, 0), (1, 1), (1, 2), (2, 0), (2, 1), (2, 2)]
TAPS_TOP = [(1, 0), (1, 1), (1, 2), (0, 0), (0, 1), (0, 2), (2, 0), (2, 1), (2, 2)]
TAPS_BOT = [(0, 0), (0, 1), (0, 2), (2, 0), (2, 1), (2, 2), (1, 0), (1, 1), (1, 2)]

_CACHE = {}


def _build():
    import concourse.tile as tile
    from concourse import mybir, bacc

    f32 = mybir.dt.float32
    f16 = mybir.dt.float16

    nc = bacc.Bacc("TRN2", target_bir_lowering=False, debug=False)
    x_d = nc.dram_tensor("x", [N_PER, C_IN, PIX], f16, kind="ExternalInput").ap()
    # host-pretransposed: [ci, half, k, co_half] (half-major, contiguous per
    # half); within each half tap k=3 (the first tap chunk 0 runs) is stored
    # first: host order TAPS_TOP.
    w_d = nc.dram_tensor("w", [C_IN, 2, KS * KS, 128], f16, kind="ExternalInput").ap()
    b_d = nc.dram_tensor("b", [C_OUT], f32, kind="ExternalInput").ap()
    y_d = nc.dram_tensor("y", [N_PER, C_OUT, PIX], f32, kind="ExternalOutput").ap()

    # host-order index of tap (kh, kw) inside a weight half (TAPS_TOP order)
    widx = {t: i for i, t in enumerate(TAPS_TOP)}

    with tile.TileContext(nc) as tc:
        with ExitStack() as ctx:
            wp = ctx.enter_context(tc.tile_pool(name="wp", bufs=1))
            xrawta = ctx.enter_context(tc.tile_pool(name="xrawta", bufs=1))
            xrawtb = ctx.enter_context(tc.tile_pool(name="xrawtb", bufs=1))
            xrawtc = ctx.enter_context(tc.tile_pool(name="xrawtc", bufs=1))
            xrawb = ctx.enter_context(tc.tile_pool(name="xrawb", bufs=1))
            # images 1-3 land whole, prefetched at kernel start with no deps:
            # their descriptors sit behind image-0's in the ring FIFOs, and all
            # input traffic completes before the first bulk output DMA can
            # starve it (big output descriptors occupy a queue ~0.5us each).
            xrawf = ctx.enter_context(tc.tile_pool(name="xrawf", bufs=N_PER - 1))
            xpadt = ctx.enter_context(tc.tile_pool(name="xpadt", bufs=4))
            xpadb = ctx.enter_context(tc.tile_pool(name="xpadb", bufs=4))
            pp = ctx.enter_context(tc.tile_pool(name="pp", bufs=6, space="PSUM"))
            op = ctx.enter_context(tc.tile_pool(name="op", bufs=2))

            # PE warmup: the HAM clock gate opens ~6us after the first PE
            # instruction; keep the PE busy with dummies until real work is
            # ready (~8.5us). wu memset goes on the gpsimd queue (earliest to
            # start); dummies shrink toward the end for a fine-grained handoff.
            wu = wp.tile([128, 448], f16)
            nc.gpsimd.memset(wu[:], 0.0)
            wups = pp.tile([128, NF], f32, tag="ps")
            for _ in range(6):
                nc.tensor.matmul(wups[:], wu[:, 0:128], wu[:], start=True, stop=True)
            for _ in range(10):
                nc.tensor.matmul(wups[:, 0:112], wu[:, 0:128], wu[:, 0:112], start=True, stop=True)

            # Weight half 0 in two pieces on the ACT ring: chunk 0's first six
            # taps' weights (192KB) gate the first matmuls; the rest follows.
            w_r = wp.tile([C_IN, 2 * KS * KS * 128], f16)
            w_r4 = w_r[:].rearrange("p (h k co) -> p h k co", h=2, k=KS * KS)
            nc.scalar.dma_start(
                w_r4[:, 0, 0:6], w_d[:, 0, 0:6].rearrange("ci k co -> ci (k co)")
            )
            nc.scalar.dma_start(
                w_r4[:, 0, 6:9], w_d[:, 0, 6:9].rearrange("ci k co -> ci (k co)")
            )

            bias_sb = wp.tile([128, 2], f32)

            for n in range(N_PER):
                if n == 0:
                    # image 0: top interior in three slices so chunk 0
                    # unblocks as early as possible
                    xrta = xrawta.tile([C_IN, XT_A * W], f16)
                    nc.sync.dma_start(xrta[:], x_d[n, :, 0 : XT_A * W])
                    xrtb = xrawtb.tile([C_IN, XT_B * W], f16)
                    nc.sync.dma_start(xrtb[:], x_d[n, :, XT_A * W : (XT_A + XT_B) * W])
                    xrtc = xrawtc.tile([C_IN, XT_C * W], f16)
                    nc.sync.dma_start(xrtc[:], x_d[n, :, (XT_A + XT_B) * W : T_ROWS * W])
                    xrb = xrawb.tile([C_IN, B_ROWS * W], f16)
                    nc.sync.dma_start(xrb[:], x_d[n, :, B0 * W : (B0 + B_ROWS) * W])
                    top_srcs = [(xrta[:], 0, XT_A), (xrtb[:], XT_A, XT_A + XT_B),
                                (xrtc[:], XT_A + XT_B, T_ROWS)]
                    bot_src = xrb[:]
                    # weight half 1 + bias queue behind the critical pieces
                    nc.scalar.dma_start(
                        w_r4[:, 1], w_d[:, 1].rearrange("ci k co -> ci (k co)")
                    )
                    nc.scalar.dma_start(bias_sb[:], b_d.rearrange("(h p) -> p h", h=2))
                else:
                    # images 1-3: one whole-image DMA, prefetched (no deps)
                    xrf = xrawf.tile([C_IN, PIX], f16)
                    nc.sync.dma_start(xrf[:], x_d[n])
                    top_srcs = [(xrf[:, 0 : T_ROWS * W], 0, T_ROWS)]
                    bot_src = xrf[:, B0 * W : (B0 + B_ROWS) * W]

                # copy A: interior at col 1 (kw=0,2 taps; even rhs offsets
                # 0/2); zero border cols 0 and 57.  copy B: interior at col 2
                # (kw=1 taps; even rhs offset 2), no borders needed.
                xptA = xpadt.tile([C_IN, T_ROWS * SW], f16)
                xptA3 = xptA[:].rearrange("p (a b) -> p a b", a=T_ROWS)
                xptB = xpadt.tile([C_IN, T_ROWS * SW], f16)
                xptB3 = xptB[:].rearrange("p (a b) -> p a b", a=T_ROWS)
                nc.vector.memset(xptA3[:, :, 0:1], 0.0)
                nc.vector.memset(xptA3[:, :, SW - 1 : SW], 0.0)
                # interleave A/B per slice: chunk 0 needs both A and B of the
                # first slice ASAP
                for (src, a0, a1) in top_srcs:
                    src3 = src.rearrange("p (a b) -> p a b", a=a1 - a0)
                    nc.vector.tensor_copy(xptA3[:, a0:a1, 1 : 1 + W], src3)
                    nc.vector.tensor_copy(xptB3[:, a0:a1, 2 : 2 + W], src3)

                xpbA = xpadb.tile([C_IN, B_ROWS * SW], f16)
                xpbA3 = xpbA[:].rearrange("p (a b) -> p a b", a=B_ROWS)
                xpbB = xpadb.tile([C_IN, B_ROWS * SW], f16)
                xpbB3 = xpbB[:].rearrange("p (a b) -> p a b", a=B_ROWS)
                nc.vector.memset(xpbA3[:, :, 0:1], 0.0)
                nc.vector.memset(xpbA3[:, :, SW - 1 : SW], 0.0)
                xrb3 = bot_src.rearrange("p (a b) -> p a b", a=B_ROWS)
                nc.vector.tensor_copy(xpbA3[:, :, 1 : 1 + W], xrb3)
                nc.vector.tensor_copy(xpbB3[:, :, 2 : 2 + W], xrb3)

                out_sb = op.tile([128, 2 * PIX], f32)
                last_img = n == N_PER - 1
                for half in range(2):
                    for rc in range(RC):
                        ps = pp.tile([128, NF], f32)
                        taps = TAPS_TOP if rc == 0 else (TAPS_BOT if rc == RC - 1 else TAPS_MID)
                        for i, (kh, kw) in enumerate(taps):
                            lhsT = w_r4[:, half, widx[(kh, kw)], :]
                            # x rows needed: output rows rc*8..rc*8+7 read
                            # x rows rc*8+kh-1 .. rc*8+kh+6 (clipped at 0/55)
                            r0 = rc * ROWS + kh - 1
                            narrow_top = rc == 0 and kh == 0       # skip out row 0
                            narrow_bot = rc == RC - 1 and kh == 2  # skip out row 55
                            if rc < 4:
                                A3, B3, base = xptA3, xptB3, 0
                            else:
                                A3, B3, base = xpbA3, xpbB3, B0
                            src3 = B3 if kw == 1 else A3
                            coff = 2 if kw == 1 else kw
                            if narrow_top:
                                rhs = src3[:, 0:ROWS - 1, coff : coff + W]
                                dst = ps[:, W:NF]
                            elif narrow_bot:
                                lr = r0 - base
                                rhs = src3[:, lr : lr + ROWS - 1, coff : coff + W]
                                dst = ps[:, 0:NARROW]
                            else:
                                lr = r0 - base
                                rhs = src3[:, lr : lr + ROWS, coff : coff + W]
                                dst = ps[:]
                            nc.tensor.matmul(
                                dst, lhsT, rhs,
                                start=(i == 0), stop=(i == KS * KS - 1),
                            )
                        # psum -> sbuf with per-channel bias add
                        lo = half * PIX + rc * NF
                        if last_img:
                            # last image: write every chunk out as produced so
                            # nothing bulky sits ahead of the tail in the DMA
                            # queues; final chunk in two halves on two rings.
                            eng = nc.sync if half == 0 else nc.gpsimd
                            if half == 1 and rc == RC - 1:
                                HNF = NF // 2
                                for piece in range(2):
                                    po = lo + piece * HNF
                                    nc.vector.tensor_scalar_add(
                                        out_sb[:, po : po + HNF],
                                        ps[:, piece * HNF : (piece + 1) * HNF],
                                        bias_sb[:, half : half + 1],
                                    )
                                    peng = nc.sync if piece == 0 else nc.gpsimd
                                    peng.dma_start(
                                        y_d[n, 128:256, rc * NF + piece * HNF : rc * NF + (piece + 1) * HNF],
                                        out_sb[:, po : po + HNF],
                                    )
                                continue
                            nc.vector.tensor_scalar_add(
                                out_sb[:, lo : lo + NF],
                                ps[:],
                                bias_sb[:, half : half + 1],
                            )
                            eng.dma_start(
                                y_d[n, half * 128 : (half + 1) * 128, rc * NF : (rc + 1) * NF],
                                out_sb[:, lo : lo + NF],
                            )
                            continue
                        nc.vector.tensor_scalar_add(
                            out_sb[:, lo : lo + NF],
                            ps[:],
                            bias_sb[:, half : half + 1],
                        )
                    if not last_img:
                        eng = nc.scalar if half == 0 else nc.gpsimd
                        eng.dma_start(
                            y_d[n, half * 128 : (half + 1) * 128, :],
                            out_sb[:, half * PIX : (half + 1) * PIX],
                        )
    nc.compile()
    return nc


def _get_nc():
    if "nc" not in _CACHE:
        _CACHE["nc"] = _build()
    return _CACHE["nc"]


def _prep_inputs(x, weight, bias):
    # fp16 on host: halves input DMA bytes and drops the on-device casts
    x = np.ascontiguousarray(
        np.asarray(x, dtype=np.float32).astype(np.float16).reshape(N_FULL, C_IN, PIX)
    )
    # [co, ci, kh, kw] -> [ci, half, k, co_half], half-major; taps within a
    # half stored in TAPS_TOP order so chunk-0's first weights DMA first.
    w4 = (
        np.transpose(np.asarray(weight, dtype=np.float32), (1, 2, 3, 0))
        .reshape(C_IN, KS * KS, 2, 128)
        .transpose(0, 2, 1, 3)
    )  # [ci, half, k(row-major), co]
    perm = [kh * KS + kw for (kh, kw) in TAPS_TOP]
    w_t = np.ascontiguousarray(w4[:, :, perm, :].astype(np.float16))
    b = np.ascontiguousarray(bias, dtype=np.float32)
    return x, w_t, b


def kernel(x, weight, bias):
    from concourse.bass_utils import run_bass_kernel_spmd

    x, w_t, b = _prep_inputs(x, weight, bias)
    nc = _get_nc()
    in_maps = [
        {"x": x[i * N_PER : (i + 1) * N_PER], "w": w_t, "b": b}
        for i in range(N_CORES)
    ]
    res = run_bass_kernel_spmd(nc, in_maps, list(range(N_CORES)))
    y = np.concatenate(
        [res.results[i]["y"].reshape(N_PER, C_OUT, H, W) for i in range(N_CORES)],
        axis=0,
    )
    return y
